# revision 7
# baseline (speedup 1.0000x reference)
"""Complex self-attention on 8 Trainium2 NeuronCores (Bass/Tile).

Reference computation (B=2, S=2048, F=1024, H=16, D=64):
    Q/K/V = complex_linear(x, W{q,k,v});  scores = Re(Q K^H) * D^-0.5
    attn = softmax(scores + mask_bias);  out = complex_linear(attn @ V, Wo)
    return stack([out_r, out_i])            # [2, B, S, F]

Sharding: 8 cores = 2 batches x 4 head-groups (4 heads each). Each core
computes its heads' Q/K/V projections, the attention, and a *partial*
output projection (contraction over its 256 features of Wo); the host
sums the 4 partials per batch and adds bo (the collective).

Complex arithmetic is folded into real matmuls by stacking (re, im)
parts along the contraction axis with host-prepped weight layouts:
    Xcat^T = [x_r^T ; x_i^T]   [2F, S]  (bf16)
and per-head tiles [128, *] carry (re 0:64, im 64:128) on the partition
axis. Projections use Karatsuba (m1 = xr@Wr, m2 = xi@Wi,
m3 = (xr+xi)@(Wr+Wi); re = m1-m2, im = m3-m1-m2): 3 matmul chains
instead of 4.  The whole matmul datapath is bf16 (PSUM accumulation is
f32), which runs at the full PE rate and halves DMA/SBUF traffic.

Q^T/K^T for the score matmuls are produced by blocked DMA transposes
(XBAR), not PE transposes: one dma transpose per (tensor, s-tile) moves
all 4 heads at once.  PSUM->SBUF staging copies run on the Activation
engine during phase 1 (it is otherwise idle there).

Softmax: no max subtraction (scaled scores have sigma~1.4, |s|<9, exp
is safe in f32); exp runs on 2-bank PSUM pairs ([128,2,512] per op) to
amortize the fixed ACT overhead; the denominator comes from an appended
ones/mask column on the V tiles, so the attn @ V matmul also yields
sum_k exp * mask; division happens per-partition in the natural [q, d]
layout.  When mask is all-ones (the reference always passes ones) the
mask multiply on V is skipped and the extra column is memset once.
"""

import sys

if "/opt/trn_rl_repo" not in sys.path:
    sys.path.insert(0, "/opt/trn_rl_repo")

import numpy as np
import ml_dtypes

B, S, F = 2, 2048, 1024
H, D = 16, 64
NCORES = 8
HL = 4           # heads per core
D2 = 2 * D       # 128 = (re|im) feature rows per head
NST = S // 128    # 16 query/key 128-tiles
GF = F // 128     # 8 contraction tiles per m-chain
HD = HL * D       # 256 columns per m-chain
WK3 = 3 * HD      # Karatsuba weight block width

BF16 = ml_dtypes.bfloat16

_CACHE = {}


def _build_program(with_bias=False, with_mask=False):
    from concourse import bass, bacc, mybir, tile

    F32 = mybir.dt.float32
    BF = mybir.dt.bfloat16
    EXP = mybir.ActivationFunctionType.Exp

    nc = bacc.Bacc("TRN2", target_bir_lowering=False, debug=False)

    xcat = nc.dram_tensor("xcat", [2 * F, S], BF, kind="ExternalInput")
    # Karatsuba weight blocks: [F, 3, HL*D] with m in {Wr, Wi, Wr+Wi}
    wq_d = nc.dram_tensor("wq", [F, WK3], BF, kind="ExternalInput")
    wk_d = nc.dram_tensor("wk", [F, WK3], BF, kind="ExternalInput")
    wv_d = nc.dram_tensor("wv", [F, WK3], BF, kind="ExternalInput")
    wor_d = nc.dram_tensor("wor", [HL * D2, F], BF, kind="ExternalInput")
    woi_d = nc.dram_tensor("woi", [HL * D2, F], BF, kind="ExternalInput")
    if with_bias:
        bq_d = nc.dram_tensor("bqrep", [128, HL * D2], BF, kind="ExternalInput")
        bk_d = nc.dram_tensor("bkrep", [128, HL * D2], BF, kind="ExternalInput")
        bv_d = nc.dram_tensor("bvrep", [128, HL * D2], BF, kind="ExternalInput")
    if with_mask:
        mask_d = nc.dram_tensor("maskcols", [128, NST], F32, kind="ExternalInput")
    out_r = nc.dram_tensor("out_r", [S, F], F32, kind="ExternalOutput")
    out_i = nc.dram_tensor("out_i", [S, F], F32, kind="ExternalOutput")

    scale = 1.0 / float(np.sqrt(D))

    with tile.TileContext(nc) as tc, nc.allow_low_precision("bf16 pipeline"):
        with (
            tc.tile_pool(name="consts", bufs=1) as cpool,
            tc.tile_pool(name="qkt", bufs=1) as qkt_pool,
            tc.tile_pool(name="vaug", bufs=1) as vaug_pool,
        ):
            if with_mask:
                mask_sb = cpool.tile([128, NST], F32)
                nc.sync.dma_start(mask_sb[:], mask_d.ap())
            if with_bias:
                bq_sb = cpool.tile([128, HL * D2], BF)
                nc.sync.dma_start(bq_sb[:], bq_d.ap())
                bk_sb = cpool.tile([128, HL * D2], BF)
                nc.sync.dma_start(bk_sb[:], bk_d.ap())
                bv_sb = cpool.tile([128, HL * D2], BF)
                nc.sync.dma_start(bv_sb[:], bv_d.ap())
            else:
                bq_sb = bk_sb = bv_sb = None

            # Resident activations: transposed Q/K and masked V (+denominator
            # column), all bf16
            qt = qkt_pool.tile([128, HL, S], BF)     # [d_ri, h, s]
            kt = qkt_pool.tile([128, HL, S], BF)
            va = vaug_pool.tile([128, HL, NST, D2 + 1], BF)  # [k, h, kt, d_ri|den]
            wor_sb = qkt_pool.tile([128, HL, F], BF, tag="wor")
            woi_sb = qkt_pool.tile([128, HL, F], BF, tag="woi")
            if not with_mask:
                # denominator column is a constant 1.0 for every key
                nc.vector.memset(va[:, :, :, D2 : D2 + 1], 1.0)

            # ---------------- Phase 1: Q,K,V projections (Karatsuba) --------
            # wv lives in its own pool so its prefetch DMAs land in SBUF that
            # does not overlap the phase-1 pools.
            wvpool = tc.alloc_tile_pool(name="wv", bufs=1)
            wv_sb = wvpool.tile([128, GF, WK3], BF)
            with (
                tc.tile_pool(name="wqk", bufs=1) as wpool,
                tc.tile_pool(name="xcol", bufs=3) as xpool,
                tc.tile_pool(name="xsum", bufs=2) as xsum_pool,
                tc.tile_pool(name="stage", bufs=4) as spool,
                tc.tile_pool(name="proj_ps", bufs=6, space="PSUM") as proj_ps,
            ):
                wq_sb = wpool.tile([128, GF, WK3], BF, tag="wq")
                wk_sb = wpool.tile([128, GF, WK3], BF, tag="wk")

                def load_w_fine(w_sb, w_d, m, g0, ng):
                    # one (m-block, g-range) chunk, issued on the ACT queue
                    c0 = m * HD
                    nc.scalar.dma_start(
                        w_sb[:, g0 : g0 + ng, c0 : c0 + HD],
                        w_d.ap()[
                            g0 * 128 : (g0 + ng) * 128, c0 : c0 + HD
                        ].rearrange("(g p) n -> p g n", p=128),
                    )

                def proj_chains(ps_pool, xlo, xhi, xs, w_sb, tag):
                    m1 = ps_pool.tile([128, HD], F32, tag="pm", name=f"{tag}_m1")
                    m2 = ps_pool.tile([128, HD], F32, tag="pm", name=f"{tag}_m2")
                    m3 = ps_pool.tile([128, HD], F32, tag="pm", name=f"{tag}_m3")
                    for m, x_sb, c0 in ((m1, xlo, 0), (m2, xhi, HD), (m3, xs, 2 * HD)):
                        for g in range(GF):
                            nc.tensor.matmul(m[:], x_sb[:, g, :],
                                             w_sb[:, g, c0 : c0 + HD],
                                             start=(g == 0), stop=(g == GF - 1))
                    return m1, m2, m3

                def hd_view(ap2d):
                    return ap2d.rearrange("p (h d) -> p h d", d=D)

                def combine_nat(m1, m2, m3, nat, tmp, c2, b_sb):
                    # nat[:, h*128+(0:64)] = m1-m2 ; nat[:, h*128+(64:128)] = m3-m1-m2
                    # c2 staging runs on ACT (TT reads at most one PSUM operand)
                    natv = nat[:].rearrange("p (h c) -> p h c", c=D2)
                    nc.scalar.copy(c2[:], m2[:])
                    nc.vector.tensor_sub(natv[:, :, 0:D], hd_view(m1[:]), hd_view(c2[:]))
                    nc.vector.tensor_sub(tmp[:], m3[:], c2[:])
                    nc.vector.tensor_sub(natv[:, :, D:D2], hd_view(tmp[:]), hd_view(m1[:]))
                    if with_bias:
                        nc.vector.tensor_add(nat[:], nat[:], b_sb[:])

                # early weight chunks so the first matmul starts ASAP
                load_w_fine(wq_sb, wq_d, 0, 0, 2)
                load_w_fine(wq_sb, wq_d, 0, 2, 2)

                for st in range(NST):
                    xlo = xpool.tile([128, GF, 128], BF, tag="xlo", name="xlo")
                    xhi = xpool.tile([128, GF, 128], BF, tag="xhi", name="xhi")
                    if st == 0:
                        # fine-grained first tiles: PE starts on the first half
                        for half in range(2):
                            nc.sync.dma_start(
                                xlo[:, 4 * half : 4 * half + 4, :],
                                xcat.ap()[
                                    512 * half : 512 * half + 512, 0:128
                                ].rearrange("(g p) m -> p g m", p=128),
                            )
                        for half in range(2):
                            nc.sync.dma_start(
                                xhi[:, 4 * half : 4 * half + 4, :],
                                xcat.ap()[
                                    F + 512 * half : F + 512 * half + 512, 0:128
                                ].rearrange("(g p) m -> p g m", p=128),
                            )
                        # stream the rest of the weights in consumption order
                        load_w_fine(wq_sb, wq_d, 0, 4, 2)
                        load_w_fine(wq_sb, wq_d, 0, 6, 2)
                        load_w_fine(wq_sb, wq_d, 1, 0, 2)
                        load_w_fine(wq_sb, wq_d, 1, 2, 2)
                        load_w_fine(wk_sb, wk_d, 0, 0, 4)
                        load_w_fine(wq_sb, wq_d, 1, 4, 2)
                        load_w_fine(wq_sb, wq_d, 1, 6, 2)
                        load_w_fine(wk_sb, wk_d, 0, 4, 4)
                        load_w_fine(wq_sb, wq_d, 2, 0, 2)
                        load_w_fine(wq_sb, wq_d, 2, 2, 2)
                        load_w_fine(wv_sb, wv_d, 0, 0, 4)
                        load_w_fine(wq_sb, wq_d, 2, 4, 2)
                        load_w_fine(wq_sb, wq_d, 2, 6, 2)
                        load_w_fine(wk_sb, wk_d, 1, 0, 4)
                        load_w_fine(wv_sb, wv_d, 0, 4, 4)
                        load_w_fine(wk_sb, wk_d, 1, 4, 4)
                        load_w_fine(wk_sb, wk_d, 2, 0, 4)
                        load_w_fine(wk_sb, wk_d, 2, 4, 4)
                        load_w_fine(wv_sb, wv_d, 1, 0, 4)
                        load_w_fine(wv_sb, wv_d, 1, 4, 4)
                        load_w_fine(wv_sb, wv_d, 2, 0, 4)
                        load_w_fine(wv_sb, wv_d, 2, 4, 4)
                    else:
                        nc.sync.dma_start(
                            xlo[:],
                            xcat.ap()[0:F, st * 128 : (st + 1) * 128].rearrange(
                                "(g p) m -> p g m", p=128
                            ),
                        )
                        nc.sync.dma_start(
                            xhi[:],
                            xcat.ap()[F : 2 * F, st * 128 : (st + 1) * 128].rearrange(
                                "(g p) m -> p g m", p=128
                            ),
                        )
                    xs = xsum_pool.tile([128, GF, 128], BF, name="xs")
                    nc.vector.tensor_add(xs[:], xlo[:], xhi[:])

                    q_m = proj_chains(proj_ps, xlo, xhi, xs, wq_sb, "q")
                    qn = spool.tile([128, HL * D2], BF, tag="nat", name="qn")
                    tmpq = spool.tile([128, HD], F32, tag="tmp", name="tmpq")
                    c2q = spool.tile([128, HD], F32, tag="c2", name="c2q")
                    combine_nat(*q_m, qn, tmpq, c2q, bq_sb)
                    nc.scalar.dma_start(
                        qt[:, :, st * 128 : (st + 1) * 128], qn[:], transpose=True
                    )
                    if st == 1:
                        # prefetch the O-projection weights behind the QKV ones
                        nc.scalar.dma_start(
                            wor_sb[:], wor_d.ap().rearrange("(h p) n -> p h n", p=128)
                        )
                        nc.scalar.dma_start(
                            woi_sb[:], woi_d.ap().rearrange("(h p) n -> p h n", p=128)
                        )

                    k_m = proj_chains(proj_ps, xlo, xhi, xs, wk_sb, "k")
                    kn = spool.tile([128, HL * D2], BF, tag="nat", name="kn")
                    tmpk = spool.tile([128, HD], F32, tag="tmp", name="tmpk")
                    c2k = spool.tile([128, HD], F32, tag="c2", name="c2k")
                    combine_nat(*k_m, kn, tmpk, c2k, bk_sb)
                    nc.scalar.dma_start(
                        kt[:, :, st * 128 : (st + 1) * 128], kn[:], transpose=True
                    )

                    # V chains share the same psum slots (freed by Q combines)
                    v_m1, v_m2, v_m3 = proj_chains(proj_ps, xlo, xhi, xs, wv_sb, "v")
                    c2v = spool.tile([128, HD], F32, tag="c2", name="c2v")
                    nc.scalar.copy(c2v[:], v_m2[:])
                    if not with_mask and not with_bias:
                        # write V directly into the augmented tile
                        tmpv = spool.tile([128, HD], F32, tag="tmp", name="tmpv")
                        nc.vector.tensor_sub(
                            va[:, :, st, 0:D], hd_view(v_m1[:]), hd_view(c2v[:])
                        )
                        nc.vector.tensor_sub(tmpv[:], v_m3[:], c2v[:])
                        nc.vector.tensor_sub(
                            va[:, :, st, D:D2], hd_view(tmpv[:]), hd_view(v_m1[:])
                        )
                    else:
                        t_re = spool.tile([128, HD], F32, tag="vt", name="t_re")
                        nc.vector.tensor_sub(t_re[:], v_m1[:], c2v[:])
                        t_im = spool.tile([128, HD], F32, tag="vt", name="t_im")
                        nc.vector.tensor_sub(t_im[:], v_m3[:], c2v[:])
                        nc.vector.tensor_sub(t_im[:], t_im[:], v_m1[:])
                        if with_bias:
                            bv_v = bv_sb[:].rearrange("p (h c) -> p h c", c=D2)
                            t_re_v = t_re[:].rearrange("p (h d) -> p h d", d=D)
                            t_im_v = t_im[:].rearrange("p (h d) -> p h d", d=D)
                            nc.vector.tensor_add(t_re_v[:], t_re_v[:], bv_v[:, :, 0:D])
                            nc.vector.tensor_add(t_im_v[:], t_im_v[:], bv_v[:, :, D:D2])
                        for h in range(HL):
                            if with_mask:
                                nc.vector.tensor_scalar_mul(
                                    va[:, h, st, 0:D],
                                    t_re[:, h * D : (h + 1) * D],
                                    mask_sb[:, st : st + 1],
                                )
                                nc.vector.tensor_scalar_mul(
                                    va[:, h, st, D:D2],
                                    t_im[:, h * D : (h + 1) * D],
                                    mask_sb[:, st : st + 1],
                                )
                                nc.vector.tensor_copy(
                                    va[:, h, st, D2 : D2 + 1], mask_sb[:, st : st + 1]
                                )
                            else:
                                nc.vector.tensor_copy(
                                    va[:, h, st, 0:D], t_re[:, h * D : (h + 1) * D]
                                )
                                nc.vector.tensor_copy(
                                    va[:, h, st, D:D2], t_im[:, h * D : (h + 1) * D]
                                )

            wvpool.release()

            # -------- Phase 2: attention (512-wide q blocks) + O-proj -------
            # Loop order: q-block outer, head inner; after all heads of a
            # q-block finish, that block's output projection runs - its PE
            # matmuls fill the ACT-bound (exp) stretches of the next block.
            QW = 512
            NBLK = QW // 128  # 4 s-tiles per q-block
            with (
                tc.tile_pool(name="atp", bufs=1) as at_pool,
                tc.tile_pool(name="p_sb", bufs=NST + 2) as p_pool,
                tc.tile_pool(name="asb", bufs=4) as a_pool,
                tc.tile_pool(name="rcp", bufs=4) as r_pool,
                tc.tile_pool(name="ost", bufs=4) as opool,
                tc.tile_pool(name="sc_ps", bufs=2, space="PSUM") as sc_ps,
                tc.tile_pool(name="a_ps", bufs=2, space="PSUM") as a_ps,
                tc.tile_pool(name="o_ps", bufs=2, space="PSUM") as o_ps,
            ):
                at = at_pool.tile([128, HL, S], BF)  # [d_ri, h, s] attn out^T

                def oproj_steps(st):
                    # deferred emission steps (one per PE matmul) for the
                    # output projection of s rows [st*128, (st+1)*128); the
                    # caller weaves them between exp-throttled score matmuls.
                    steps = []
                    osb_r = opool.tile([128, 2, 512], F32, tag="ost")
                    osb_i = opool.tile([128, 2, 512], F32, tag="ost")
                    for fo in range(2):
                        opr = o_ps.tile([128, 512], F32, tag="o")
                        opi = o_ps.tile([128, 512], F32, tag="o")

                        def mk_mm(ps, w_sb, h2, fo=fo):
                            def go():
                                nc.tensor.matmul(
                                    ps[:], at[:, h2, st * 128 : (st + 1) * 128],
                                    w_sb[:, h2, fo * 512 : (fo + 1) * 512],
                                    start=(h2 == 0), stop=(h2 == HL - 1),
                                )
                            return go

                        for h2 in range(HL):
                            steps.append(mk_mm(opr, wor_sb, h2))
                            steps.append(mk_mm(opi, woi_sb, h2))

                        def mk_cp(opr=opr, opi=opi, fo=fo):
                            def go():
                                nc.vector.tensor_copy(osb_r[:, fo, :], opr[:])
                                nc.vector.tensor_copy(osb_i[:, fo, :], opi[:])
                            return go

                        steps.append(mk_cp())

                    def mk_store():
                        def go():
                            for osb, dram in ((osb_r, out_r), (osb_i, out_i)):
                                nc.gpsimd.dma_start(
                                    dram.ap()[st * 128 : (st + 1) * 128, :].rearrange(
                                        "p (f n) -> p f n", f=2
                                    ),
                                    osb[:],
                                )
                        return go

                    steps.append(mk_store())
                    return steps

                for qbb in range(S // QW):
                    q0 = qbb * QW
                    for h in range(HL):
                        steps = (
                            oproj_steps((qbb - 1) * NBLK + h) if qbb > 0 else []
                        )
                        p_tiles = []
                        for pair in range(NST // 2):
                            sps = sc_ps.tile([128, 2, 512], F32, tag="sc")
                            for j in range(2):
                                kk = (2 * pair + j) * 128
                                nc.tensor.matmul(
                                    sps[:, j, :],
                                    kt[:, h, kk : kk + 128],
                                    qt[:, h, q0 : q0 + 512],
                                )
                            pt = p_pool.tile([128, 2, 512], BF, tag="p")
                            nc.scalar.activation(pt[:], sps[:], EXP, scale=scale)
                            p_tiles.append(pt)
                            # weave oproj steps into the exp-throttled stretch
                            if pair >= 1:
                                for _ in range(3):
                                    if steps:
                                        steps.pop(0)()
                        for s_fn in steps:
                            s_fn()
                        for qs in range(QW // 128):
                            aps = a_ps.tile([128, D2 + 1], F32, tag="a")
                            for ktile in range(NST):
                                nc.tensor.matmul(
                                    aps[:],
                                    p_tiles[ktile // 2][
                                        :, ktile % 2, qs * 128 : (qs + 1) * 128
                                    ],
                                    va[:, h, ktile, :],
                                    start=(ktile == 0), stop=(ktile == NST - 1),
                                )
                            rcp = r_pool.tile([128, 1], F32, tag="r")
                            nc.vector.reciprocal(rcp[:], aps[:, D2 : D2 + 1])
                            asb = a_pool.tile([128, D2], BF, tag="asb")
                            nc.vector.tensor_scalar_mul(asb[:], aps[:, 0:D2], rcp[:])
                            nc.sync.dma_start(
                                at[:, h, q0 + qs * 128 : q0 + (qs + 1) * 128],
                                asb[:],
                                transpose=True,
                            )
                            # last q-block: its own oproj blocks become ready
                            # one-by-one as the final head's at slices land
                            if h == HL - 1 and qbb == S // QW - 1:
                                for s_fn in oproj_steps(qbb * NBLK + qs):
                                    s_fn()

    nc.compile()
    return nc


def _get_program(with_bias=False, with_mask=False):
    key = f"nc_bias{with_bias}_mask{with_mask}"
    if key not in _CACHE:
        _CACHE[key] = _build_program(with_bias=with_bias, with_mask=with_mask)
    return _CACHE[key]


def _prep_core_inputs(inputs, core, with_bias, with_mask):
    """Host-side shard prep for one core (batch b, heads h0..h0+3)."""
    f32 = np.float32
    b = core // (NCORES // B)
    h0 = (core % (NCORES // B)) * HL
    hs = slice(h0 * D, (h0 + HL) * D)  # feature slice of this core's heads

    xr = np.asarray(inputs["x_r"][b], dtype=f32)
    xi = np.asarray(inputs["x_i"][b], dtype=f32)
    xcat = np.concatenate([xr.T, xi.T], axis=0)  # [2F, S]
    xcat = np.ascontiguousarray(xcat.astype(BF16))

    def wstack(wr, wi):
        # Karatsuba blocks [F, 3, HL*D]: m0 = Wr, m1 = Wi, m2 = Wr+Wi
        wr = np.asarray(wr, dtype=f32)[:, hs]
        wi = np.asarray(wi, dtype=f32)[:, hs]
        w = np.stack([wr, wi, wr + wi], axis=1)  # [F, 3, HL*D]
        return np.ascontiguousarray(w.reshape(F, 3 * HL * D).astype(BF16))

    def brep(br, bi):
        br = np.asarray(br, dtype=f32)[hs].reshape(HL, D)
        bi = np.asarray(bi, dtype=f32)[hs].reshape(HL, D)
        bcat = np.concatenate([br, bi], axis=1).reshape(HL * D2)
        return np.ascontiguousarray(
            np.broadcast_to(bcat, (128, HL * D2)).astype(BF16)
        )

    def wostack(wor, woi):
        # rows r<64 -> wo_top[d], r>=64 -> wo_bot[d]  per head
        wor = np.asarray(wor, dtype=f32)[hs].reshape(HL, D, F)
        woi = np.asarray(woi, dtype=f32)[hs].reshape(HL, D, F)
        w = np.empty((HL, D2, F), dtype=f32)
        w[:, :D] = wor
        w[:, D:] = woi
        return np.ascontiguousarray(w.reshape(HL * D2, F).astype(BF16))

    out = {
        "xcat": xcat,
        "wq": wstack(inputs["Wq_r"], inputs["Wq_i"]),
        "wk": wstack(inputs["Wk_r"], inputs["Wk_i"]),
        "wv": wstack(inputs["Wv_r"], inputs["Wv_i"]),
        "wor": wostack(inputs["Wo_r"], -np.asarray(inputs["Wo_i"], dtype=f32)),
        "woi": wostack(inputs["Wo_i"], inputs["Wo_r"]),
    }
    if with_bias:
        out["bqrep"] = brep(inputs["bq_r"], inputs["bq_i"])
        out["bkrep"] = brep(inputs["bk_r"], inputs["bk_i"])
        out["bvrep"] = brep(inputs["bv_r"], inputs["bv_i"])
    if with_mask:
        mask = np.asarray(inputs["mask"][b], dtype=f32)
        out["maskcols"] = np.ascontiguousarray(mask.reshape(NST, 128).T)
    return out


def kernel(_trace=False, _trace_kwargs=None, **inputs):
    from concourse.bass_utils import run_bass_kernel_spmd

    with_bias = bool(any(
        np.any(np.asarray(inputs[k]))
        for k in ("bq_r", "bq_i", "bk_r", "bk_i", "bv_r", "bv_i")
    ))
    with_mask = not bool(np.all(np.asarray(inputs["mask"]) == 1.0))
    nc = _get_program(with_bias=with_bias, with_mask=with_mask)
    in_maps = [
        _prep_core_inputs(inputs, c, with_bias, with_mask) for c in range(NCORES)
    ]
    res = run_bass_kernel_spmd(
        nc, in_maps, core_ids=list(range(NCORES)),
        trace=_trace, **(_trace_kwargs or {}),
    )
    _CACHE["last_results"] = res

    bo_r = np.asarray(inputs["bo_r"], dtype=np.float32)
    bo_i = np.asarray(inputs["bo_i"], dtype=np.float32)
    out = np.empty((2, B, S, F), dtype=np.float32)
    cpb = NCORES // B
    for b in range(B):
        cores = range(b * cpb, (b + 1) * cpb)
        out[0, b] = sum(res.results[c]["out_r"] for c in cores) + bo_r
        out[1, b] = sum(res.results[c]["out_i"] for c in cores) + bo_i
    return out
